# revision 15
# baseline (speedup 1.0000x reference)
"""Trainium2 Bass kernel for BrickVectorEdgeModel (GNN message passing).

Reference computation (per batch element b of 8):
  f  = relu(relu(x @ Wa + ba) @ Wb + bb)            # node MLP, x: [128, 256]
  e[i, j] = cat(f[j], f[i])                         # pairwise concat
  h1 = relu(e @ Wca + bca)                          # decomposed: G[j] + H[i]
  h2 = relu(h1 @ Wcb + bcb)
  h3 = relu(h2 @ Wcc + bcc)
  out[i, j] = h3 @ Wo + bo                          # [128, 128, 2]

Sharding: data-parallel over batch, one batch element per NeuronCore (8 cores).

Device kernel works in transposed activation layout [feat (partitions), cols]:
each layer is out_T[fo, col] = sum_k W[k, fo] * act_T[k, col], i.e.
matmul(psum, lhsT=W_chunk, rhs=actT_chunk), so activations never need an
on-chip transpose. The first edge layer is decomposed:
  h1_T[:, (i, j)] = relu(G_T[:, j] + (H_T[:, i] + bca))
built with WIDE broadcast-AP ops: one DVE tensor_tensor add (512 cols, G
broadcast over i, H broadcast over j) + one GpSimd relu per feature chunk.

The out layer ([512 -> 2] projection) uses 4-way PE column tiling: the four
K-chunk matmuls run CONCURRENTLY in disjoint 32-column groups of the PE
array (tile_position), ~2.8x faster than sequential; the four partial sums
land at psum partitions {0,32,64,96} and are combined on DVE/GpSimd.

Biases are folded into the PSUM-drain activations (per-partition bias), not
K=1 ones-matmuls.

All matmuls run in bf16 with fp32 PSUM accumulation.
"""

import numpy as np
import ml_dtypes

import concourse.bass as bass
import concourse.mybir as mybir
import concourse.tile as tile
from concourse import bacc
from concourse.bass_utils import run_bass_kernel_spmd

BF16 = mybir.dt.bfloat16
F32 = mybir.dt.float32

B = 8          # batch == number of cores
N = 128        # bricks per model (nodes)
D_IN = 256     # input feature dim
H = 512        # hidden dim
KA = D_IN // 128   # 2 input-feature chunks
C = H // 128       # 4 hidden-feature chunks
IG = 4             # i-values per group (4 * 128 cols = 512 = one PSUM bank)
NG = N // IG       # 32 groups

PK1 = 16 + 512 + 2048                    # ba | xT | Wa
PK2A = 32 + 4096                         # bb | bca | Wb   (sync queue)
PK2B = 2 * 4096                          # Wcaj | Wcai     (scalar queue)
PK3 = 48 + 2 * 4096 + 16                 # bcb | bcc | bo | Wcb | Wcc | Wo

LAST_RESULTS = None


def _build_nc() -> bass.Bass:
    # Bacc (not raw Bass): its compile pass legalizes multi-wait sync_info
    # into forms walrus codegen accepts.
    nc = bacc.Bacc("TRN2", target_bir_lowering=False)

    pk1 = nc.dram_tensor("pk1", [128, PK1], mybir.dt.uint8, kind="ExternalInput")
    pk2a = nc.dram_tensor("pk2a", [128, PK2A], mybir.dt.uint8, kind="ExternalInput")
    pk2b = nc.dram_tensor("pk2b", [128, PK2B], mybir.dt.uint8, kind="ExternalInput")
    pk3 = nc.dram_tensor("pk3", [128, PK3], mybir.dt.uint8, kind="ExternalInput")

    # Output in transposed layout [2, i, j]; host transposes to [i, j, 2].
    out = nc.dram_tensor("out", [2, N, N], F32, kind="ExternalOutput")

    relu = mybir.ActivationFunctionType.Relu
    ident = mybir.ActivationFunctionType.Identity
    add_op = mybir.AluOpType.add
    max_op = mybir.AluOpType.max

    with tile.TileContext(nc) as tc:
        with (
            tc.tile_pool(name="consts", bufs=1) as consts,
            tc.tile_pool(name="work", bufs=8) as work,
            tc.tile_pool(name="outp", bufs=6) as outp,
            tc.tile_pool(name="psmid", bufs=8, space="PSUM") as psmid,
        ):
            # ---- load constants: 3 packed buffers, 1 DMA dispatch each ---------
            # (each DMA dispatch costs ~600 ns on the queue engine; 14 serial
            # dispatches delayed the first matmul by ~4 us). pack1 (xT/Wa/ba)
            # unblocks the node MLP; pack2/pack3 go on the otherwise-unused
            # GpSimd queue so they flow in parallel with pack1.
            pk1_sb = consts.tile([128, PK1], mybir.dt.uint8, tag="pk1_sb")
            pk2a_sb = consts.tile([128, PK2A], mybir.dt.uint8, tag="pk2a_sb")
            pk2b_sb = consts.tile([128, PK2B], mybir.dt.uint8, tag="pk2b_sb")
            pk3_sb = consts.tile([128, PK3], mybir.dt.uint8, tag="pk3_sb")
            nc.sync.dma_start(out=pk1_sb, in_=pk1[:])
            nc.sync.dma_start(out=pk2a_sb, in_=pk2a[:])
            nc.scalar.dma_start(out=pk2b_sb, in_=pk2b[:])
            nc.gpsimd.dma_start(out=pk3_sb, in_=pk3[:])

            def view(pk, off, nbytes, dt, shape):
                v = pk[:, off:off + nbytes].bitcast(dt)
                if len(shape) == 3:
                    v = v.rearrange("p (a b) -> p a b", a=shape[1])
                return v

            ba_sb = view(pk1_sb, 0, 16, F32, [128, C])
            xT_sb = view(pk1_sb, 16, 512, BF16, [128, KA, N])
            wa_sb = view(pk1_sb, 528, 2048, BF16, [128, KA, H])
            bb_sb = view(pk2a_sb, 0, 16, F32, [128, C])
            bca_sb = view(pk2a_sb, 16, 16, F32, [128, C])
            wb_sb = view(pk2a_sb, 32, 4096, BF16, [128, C, H])
            wcaj_sb = view(pk2b_sb, 0, 4096, BF16, [128, C, H])
            wcai_sb = view(pk2b_sb, 4096, 4096, BF16, [128, C, H])
            bcb_sb = view(pk3_sb, 0, 16, F32, [128, C])
            bcc_sb = view(pk3_sb, 16, 16, F32, [128, C])
            bo_sb = view(pk3_sb, 32, 4, F32, [128, 1])[0:2, :]
            wcb_sb = view(pk3_sb, 48, 4096, BF16, [128, C, H])
            wcc_sb = view(pk3_sb, 4144, 4096, BF16, [128, C, H])
            wo_sb = view(pk3_sb, 8240, 16, BF16, [128, C, 2])

            # ---- PE warmup: dummy matmuls on a memset tile while the const
            # DMAs are in flight, so the PE reaches full p-state (~2.4 GHz)
            # before the real preamble matmuls instead of running them ~3x
            # slow from cold.
            warm_sb = consts.tile([128, H], BF16, tag="warm_sb")
            nc.vector.memset(warm_sb, 0.25)
            pwarm = psmid.tile([128, IG * N], F32, tag="pst", name="pwarm")
            for _ in range(10):
                nc.tensor.matmul(pwarm, warm_sb[:, :128], warm_sb,
                                 start=True, stop=True)

            # ---- node MLP: f2_T = relu(Wb_T @ relu(Wa_T @ x_T + ba) + bb) ------
            # Per-fo psum + drain-with-bias so drains overlap later fo matmuls.
            def node_layer(w_sb, in_sb, kc, out_sb, bias_sb):
                for fo in range(C):
                    pst = psmid.tile([128, N], F32, tag="pst", name="pnode")
                    for k in range(kc):
                        nc.tensor.matmul(
                            pst, w_sb[:, k, fo * 128:(fo + 1) * 128],
                            in_sb[:, k, :],
                            start=(k == 0), stop=(k == kc - 1),
                        )
                    nc.scalar.activation(out_sb[:, fo, :], pst, relu,
                                         bias=bias_sb[:, fo:fo + 1])

            f1_sb = consts.tile([128, C, N], BF16, tag="f1_sb")
            node_layer(wa_sb, xT_sb, KA, f1_sb, ba_sb)
            for _ in range(4):
                nc.tensor.matmul(pwarm, warm_sb[:, :128], warm_sb,
                                 start=True, stop=True)
            f2_sb = consts.tile([128, C, N], BF16, tag="f2_sb")
            node_layer(wb_sb, f1_sb, C, f2_sb, bb_sb)

            # ---- G_T = Wcaj_T @ f2_T ; H'_T = Wcai_T @ f2_T + bca --------------
            gt_sb = consts.tile([128, C, N], BF16, tag="gt_sb")
            ht_sb = consts.tile([128, C, N], F32, tag="ht_sb")
            for fo in range(C):
                pst = psmid.tile([128, N], F32, tag="pst", name="pgh")
                for k in range(C):
                    nc.tensor.matmul(
                        pst, wcaj_sb[:, k, fo * 128:(fo + 1) * 128], f2_sb[:, k, :],
                        start=(k == 0), stop=(k == C - 1),
                    )
                nc.scalar.copy(gt_sb[:, fo, :], pst)
                pst2 = psmid.tile([128, N], F32, tag="pst", name="pgh")
                for k in range(C):
                    nc.tensor.matmul(
                        pst2, wcai_sb[:, k, fo * 128:(fo + 1) * 128], f2_sb[:, k, :],
                        start=(k == 0), stop=(k == C - 1),
                    )
                nc.scalar.activation(ht_sb[:, fo, :], pst2, ident,
                                     bias=bca_sb[:, fo:fo + 1])

            # ---- h1 build: per-(chunk, i) DVE tensor_scalar add+relu -----------
            # h1[:, c, il, j] = relu(G[:, c, j] + H'[:, c, g*IG+il]).
            # The per-i addend forces 128-col ops (one per-partition scalar
            # per op); inner-dim broadcast APs measure 2-8x slower, GpSimd
            # measures ~15 ns/col - so 16 narrow DVE ops is the best shape.
            def build_h1(g, eng=None):
                h1 = work.tile([128, C, IG, N], BF16, tag="h1_sb")
                for c in range(C):
                    for il in range(IG):
                        if eng == "act":
                            nc.scalar.activation(
                                h1[:, c, il], gt_sb[:, c, :], relu,
                                bias=ht_sb[:, c, g * IG + il:g * IG + il + 1],
                            )
                        else:
                            nc.vector.tensor_scalar(
                                h1[:, c, il], gt_sb[:, c, :],
                                ht_sb[:, c, g * IG + il:g * IG + il + 1],
                                0.0, add_op, max_op,
                            )
                return h1

            # ---- edge layer helper: 16 matmuls + 4 drains ----------------------
            def edge_layer(w_sb, in_sb, out_sb, bias_sb, drain):
                for fo in range(C):
                    pst = psmid.tile([128, IG * N], F32, tag="pst", name="pedge")
                    for k in range(C):
                        nc.tensor.matmul(
                            pst, w_sb[:, k, fo * 128:(fo + 1) * 128],
                            in_sb[:, k],
                            start=(k == 0), stop=(k == C - 1),
                        )
                    drain(fo, pst, out_sb, bias_sb)

            def drain_act(fo, pst, out_sb, bias_sb):
                nc.scalar.activation(out_sb[:, fo], pst, relu,
                                     bias=bias_sb[:, fo:fo + 1])

            def drain_mixed(fo, pst, out_sb, bias_sb):
                # even fo -> DVE, odd fo -> ACT: the two h3 chunks drain in
                # parallel so the following out matmuls start sooner.
                if fo % 2 == 0:
                    nc.vector.tensor_scalar(
                        out_sb[:, fo], pst, bias_sb[:, fo:fo + 1],
                        0.0, add_op, max_op,
                    )
                else:
                    drain_act(fo, pst, out_sb, bias_sb)

            # ---- out layer: 4-way col-tiled matmuls + partial-sum combine -----
            def emit_out(g, h3_sb):
                # 2-way col tiling: K-chunks {0,1} accumulate at psum
                # partitions 0-1 (array cols 0-31), chunks {2,3} at 32-33
                # (array cols 32-63); the two streams overlap in the PE.
                # Interleaved emission so the second stream issues while the
                # first is still streaming (pending-zero marking is
                # per-partition, so the interleaved start flags are safe).
                pso = psmid.tile([128, IG * N], F32, tag="pst", name="pedge")
                nc.tensor.matmul(pso[0:2, :], wo_sb[:, 0, :], h3_sb[:, 0],
                                 start=True, stop=False, tile_position=(0, 0),
                                 skip_group_check=True)
                nc.tensor.matmul(pso[32:34, :], wo_sb[:, 2, :], h3_sb[:, 2],
                                 start=True, stop=False, tile_position=(0, 32),
                                 skip_group_check=True)
                nc.tensor.matmul(pso[0:2, :], wo_sb[:, 1, :], h3_sb[:, 1],
                                 start=False, stop=True, tile_position=(0, 0),
                                 skip_group_check=True)
                nc.tensor.matmul(pso[32:34, :], wo_sb[:, 3, :], h3_sb[:, 3],
                                 start=False, stop=True, tile_position=(0, 32),
                                 skip_group_check=True)
                # ACT drains both partials immediately (frees the psum bank
                # without waiting on the DVE queue); bo is folded into s1.
                s1 = outp.tile([2, IG * N], F32, tag="s1_sb")
                nc.scalar.activation(s1, pso[0:2, :], ident, bias=bo_sb)
                s2 = outp.tile([2, IG * N], F32, tag="s2_sb")
                nc.scalar.copy(s2, pso[32:34, :])
                return s1, s2

            def emit_out_final(g, s1, s2):
                o_sb = outp.tile([2, IG * N], F32, tag="o_sb")
                nc.vector.tensor_tensor(o_sb, s1, s2, add_op)
                o_v = o_sb.rearrange("p (i j) -> p i j", i=IG)
                nc.sync.dma_start(out=out[:, g * IG:(g + 1) * IG, :], in_=o_v)

            # ---- main pipeline -------------------------------------------------
            # Depth-2 software pipeline: PE order per iteration g is
            #   cb(g+1) -> out(g-1) -> cc(g)
            # so the PE never waits on a drain issued in the same iteration:
            # cc(g) consumes h2(g) drained during the previous iteration, and
            # out(g-1) consumes h3(g-1) drained during cb(g)/cb(g+1).
            h1_t = [None] * NG
            h2_t = [None] * NG
            h3_t = [None] * NG

            h1_t[0] = build_h1(0)
            h1_t[1] = build_h1(1, eng="act")
            h1_t[2] = build_h1(2)
            h2_t[0] = work.tile([128, C, IG, N], BF16, name="h2_sb", tag="h2_sb")
            edge_layer(wcb_sb, h1_t[0], h2_t[0], bcb_sb, drain_act)

            s_prev = None
            for g in range(NG):
                if g + 1 < NG:
                    h2_t[g + 1] = work.tile([128, C, IG, N], BF16, name="h2_sb", tag="h2_sb")
                    edge_layer(wcb_sb, h1_t[g + 1], h2_t[g + 1], bcb_sb,
                               drain_act)
                    h1_t[g + 1] = None
                if g >= 1:
                    s_cur = emit_out(g - 1, h3_t[g - 1])
                    h3_t[g - 1] = None
                if g + 3 < NG:
                    h1_t[g + 3] = build_h1(g + 3)
                h3_t[g] = work.tile([128, C, IG, N], BF16, name="h3_sb", tag="h3_sb")
                edge_layer(wcc_sb, h2_t[g], h3_t[g], bcc_sb, drain_mixed)
                h2_t[g] = None
                if s_prev is not None:
                    emit_out_final(g - 2, *s_prev)
                s_prev = s_cur if g >= 1 else None

            emit_out_final(NG - 2, *s_prev)
            # Last group: sequential accumulation + one biased ACT drain is
            # ~1.4 us shorter on the tail critical path than the tiled
            # matmuls + 3-op combine.
            pso = psmid.tile([128, IG * N], F32, tag="pst", name="pedge")
            for k in range(C):
                nc.tensor.matmul(pso[0:2, :], wo_sb[:, k, :],
                                 h3_t[NG - 1][:, k],
                                 start=(k == 0), stop=(k == C - 1))
            o_sb = outp.tile([2, IG * N], F32, tag="o_sb")
            nc.scalar.activation(o_sb, pso[0:2, :], ident, bias=bo_sb)
            o_v = o_sb.rearrange("p (i j) -> p i j", i=IG)
            nc.sync.dma_start(out=out[:, (NG - 1) * IG:NG * IG, :], in_=o_v)

    nc.compile()
    return nc


def _pack_w(w: np.ndarray) -> np.ndarray:
    """[K, F] f32 -> [128, K//128, F] bf16 so W[k, f] = packed[k % 128, k // 128, f]."""
    k, f = w.shape
    return np.ascontiguousarray(
        w.reshape(k // 128, 128, f).transpose(1, 0, 2)
    ).astype(ml_dtypes.bfloat16)


def _pack_b(b: np.ndarray) -> np.ndarray:
    """[F] f32 -> [128, F//128] f32 so b[f] = packed[f % 128, f // 128]."""
    return np.ascontiguousarray(b.reshape(-1, 128).T).astype(np.float32)


def kernel(brick_vectors, Wa, ba, Wb, bb, Wca, bca, Wcb, bcb, Wcc, bcc, Wo, bo):
    global LAST_RESULTS
    brick_vectors = np.asarray(brick_vectors, dtype=np.float32)

    def u8(a):
        return np.ascontiguousarray(a).view(np.uint8).reshape(128, -1)

    bo_col = np.zeros((128, 1), dtype=np.float32)
    bo_col[:2, 0] = np.asarray(bo, dtype=np.float32).ravel()
    pk2a = np.concatenate([
        u8(_pack_b(np.asarray(bb))), u8(_pack_b(np.asarray(bca))),
        u8(_pack_w(np.asarray(Wb))),
    ], axis=1)
    pk2b = np.concatenate([
        u8(_pack_w(np.asarray(Wca)[:H])), u8(_pack_w(np.asarray(Wca)[H:])),
    ], axis=1)
    pk3 = np.concatenate([
        u8(_pack_b(np.asarray(bcb))), u8(_pack_b(np.asarray(bcc))),
        u8(bo_col), np.zeros((128, 12), dtype=np.uint8),
        u8(_pack_w(np.asarray(Wcb))), u8(_pack_w(np.asarray(Wcc))),
        u8(_pack_w(np.asarray(Wo))),
    ], axis=1)
    shared = {"pk2a": pk2a, "pk2b": pk2b, "pk3": pk3}

    ba_u8 = u8(_pack_b(np.asarray(ba)))
    wa_u8 = u8(_pack_w(np.asarray(Wa)))
    in_maps = []
    for b in range(B):
        xt = _pack_w(brick_vectors[b].T.astype(np.float32))  # [128, KA, N] bf16
        pk1 = np.concatenate([ba_u8, u8(xt), wa_u8], axis=1)
        in_maps.append({"pk1": pk1, **shared})

    nc = _build_nc()
    res = run_bass_kernel_spmd(nc, in_maps, core_ids=list(range(B)))
    LAST_RESULTS = res

    out = np.empty((B, N, N, 2), dtype=np.float32)
    for b in range(B):
        out[b] = res.results[b]["out"].transpose(1, 2, 0)
    return out


# revision 16
# speedup vs baseline: 1.0131x; 1.0131x over previous
"""Trainium2 Bass kernel for BrickVectorEdgeModel (GNN message passing).

Reference computation (per batch element b of 8):
  f  = relu(relu(x @ Wa + ba) @ Wb + bb)            # node MLP, x: [128, 256]
  e[i, j] = cat(f[j], f[i])                         # pairwise concat
  h1 = relu(e @ Wca + bca)                          # decomposed: G[j] + H[i]
  h2 = relu(h1 @ Wcb + bcb)
  h3 = relu(h2 @ Wcc + bcc)
  out[i, j] = h3 @ Wo + bo                          # [128, 128, 2]

Sharding: data-parallel over batch, one batch element per NeuronCore (8 cores).

Device kernel works in transposed activation layout [feat (partitions), cols]:
each layer is out_T[fo, col] = sum_k W[k, fo] * act_T[k, col], i.e.
matmul(psum, lhsT=W_chunk, rhs=actT_chunk), so activations never need an
on-chip transpose. The first edge layer is decomposed:
  h1_T[:, (i, j)] = relu(G_T[:, j] + (H_T[:, i] + bca))
built with WIDE broadcast-AP ops: one DVE tensor_tensor add (512 cols, G
broadcast over i, H broadcast over j) + one GpSimd relu per feature chunk.

The out layer ([512 -> 2] projection) uses 4-way PE column tiling: the four
K-chunk matmuls run CONCURRENTLY in disjoint 32-column groups of the PE
array (tile_position), ~2.8x faster than sequential; the four partial sums
land at psum partitions {0,32,64,96} and are combined on DVE/GpSimd.

Biases are folded into the PSUM-drain activations (per-partition bias), not
K=1 ones-matmuls.

All matmuls run in bf16 with fp32 PSUM accumulation.
"""

import numpy as np
import ml_dtypes

import concourse.bass as bass
import concourse.mybir as mybir
import concourse.tile as tile
from concourse import bacc
from concourse.bass_utils import run_bass_kernel_spmd

BF16 = mybir.dt.bfloat16
F32 = mybir.dt.float32

B = 8          # batch == number of cores
N = 128        # bricks per model (nodes)
D_IN = 256     # input feature dim
H = 512        # hidden dim
KA = D_IN // 128   # 2 input-feature chunks
C = H // 128       # 4 hidden-feature chunks
IG = 4             # i-values per group (4 * 128 cols = 512 = one PSUM bank)
NG = N // IG       # 32 groups

PK1 = 16 + 512 + 2048                    # ba | xT | Wa
PK2A = 32 + 4096                         # bb | bca | Wb   (sync queue)
PK2B = 2 * 4096                          # Wcaj | Wcai     (scalar queue)
PK3A = 48 + 4096                         # bcb | bcc | bo | Wcb (sync, 2nd)
PK3B = 4096 + 16                         # Wcc | Wo        (scalar, 2nd)

LAST_RESULTS = None


def _build_nc() -> bass.Bass:
    # Bacc (not raw Bass): its compile pass legalizes multi-wait sync_info
    # into forms walrus codegen accepts.
    nc = bacc.Bacc("TRN2", target_bir_lowering=False)

    pk1 = nc.dram_tensor("pk1", [128, PK1], mybir.dt.uint8, kind="ExternalInput")
    pk2a = nc.dram_tensor("pk2a", [128, PK2A], mybir.dt.uint8, kind="ExternalInput")
    pk2b = nc.dram_tensor("pk2b", [128, PK2B], mybir.dt.uint8, kind="ExternalInput")
    pk3a = nc.dram_tensor("pk3a", [128, PK3A], mybir.dt.uint8, kind="ExternalInput")
    pk3b = nc.dram_tensor("pk3b", [128, PK3B], mybir.dt.uint8, kind="ExternalInput")

    # Output in transposed layout [2, i, j]; host transposes to [i, j, 2].
    out = nc.dram_tensor("out", [2, N, N], F32, kind="ExternalOutput")

    relu = mybir.ActivationFunctionType.Relu
    ident = mybir.ActivationFunctionType.Identity
    add_op = mybir.AluOpType.add
    max_op = mybir.AluOpType.max

    with tile.TileContext(nc) as tc:
        with (
            tc.tile_pool(name="consts", bufs=1) as consts,
            tc.tile_pool(name="work", bufs=8) as work,
            tc.tile_pool(name="outp", bufs=6) as outp,
            tc.tile_pool(name="psmid", bufs=8, space="PSUM") as psmid,
        ):
            # ---- load constants: 3 packed buffers, 1 DMA dispatch each ---------
            # (each DMA dispatch costs ~600 ns on the queue engine; 14 serial
            # dispatches delayed the first matmul by ~4 us). pack1 (xT/Wa/ba)
            # unblocks the node MLP; pack2/pack3 go on the otherwise-unused
            # GpSimd queue so they flow in parallel with pack1.
            pk1_sb = consts.tile([128, PK1], mybir.dt.uint8, tag="pk1_sb")
            pk2a_sb = consts.tile([128, PK2A], mybir.dt.uint8, tag="pk2a_sb")
            pk2b_sb = consts.tile([128, PK2B], mybir.dt.uint8, tag="pk2b_sb")
            pk3a_sb = consts.tile([128, PK3A], mybir.dt.uint8, tag="pk3a_sb")
            pk3b_sb = consts.tile([128, PK3B], mybir.dt.uint8, tag="pk3b_sb")
            nc.sync.dma_start(out=pk1_sb, in_=pk1[:])
            nc.scalar.dma_start(out=pk2a_sb, in_=pk2a[:])
            nc.gpsimd.dma_start(out=pk2b_sb, in_=pk2b[:])
            nc.sync.dma_start(out=pk3a_sb, in_=pk3a[:])
            nc.scalar.dma_start(out=pk3b_sb, in_=pk3b[:])

            def view(pk, off, nbytes, dt, shape):
                v = pk[:, off:off + nbytes].bitcast(dt)
                if len(shape) == 3:
                    v = v.rearrange("p (a b) -> p a b", a=shape[1])
                return v

            ba_sb = view(pk1_sb, 0, 16, F32, [128, C])
            xT_sb = view(pk1_sb, 16, 512, BF16, [128, KA, N])
            wa_sb = view(pk1_sb, 528, 2048, BF16, [128, KA, H])
            bb_sb = view(pk2a_sb, 0, 16, F32, [128, C])
            bca_sb = view(pk2a_sb, 16, 16, F32, [128, C])
            wb_sb = view(pk2a_sb, 32, 4096, BF16, [128, C, H])
            wcaj_sb = view(pk2b_sb, 0, 4096, BF16, [128, C, H])
            wcai_sb = view(pk2b_sb, 4096, 4096, BF16, [128, C, H])
            bcb_sb = view(pk3a_sb, 0, 16, F32, [128, C])
            bcc_sb = view(pk3a_sb, 16, 16, F32, [128, C])
            bo_sb = view(pk3a_sb, 32, 4, F32, [128, 1])[0:2, :]
            wcb_sb = view(pk3a_sb, 48, 4096, BF16, [128, C, H])
            wcc_sb = view(pk3b_sb, 0, 4096, BF16, [128, C, H])
            wo_sb = view(pk3b_sb, 4096, 16, BF16, [128, C, 2])

            # ---- PE warmup: dummy matmuls on a memset tile while the const
            # DMAs are in flight, so the PE reaches full p-state (~2.4 GHz)
            # before the real preamble matmuls instead of running them ~3x
            # slow from cold.
            warm_sb = consts.tile([128, H], BF16, tag="warm_sb")
            nc.vector.memset(warm_sb, 0.25)
            pwarm = psmid.tile([128, IG * N], F32, tag="pst", name="pwarm")
            for _ in range(10):
                nc.tensor.matmul(pwarm, warm_sb[:, :128], warm_sb,
                                 start=True, stop=True)

            # ---- node MLP: f2_T = relu(Wb_T @ relu(Wa_T @ x_T + ba) + bb) ------
            # Per-fo psum + drain-with-bias so drains overlap later fo matmuls.
            def node_layer(w_sb, in_sb, kc, out_sb, bias_sb):
                for fo in range(C):
                    pst = psmid.tile([128, N], F32, tag="pst", name="pnode")
                    for k in range(kc):
                        nc.tensor.matmul(
                            pst, w_sb[:, k, fo * 128:(fo + 1) * 128],
                            in_sb[:, k, :],
                            start=(k == 0), stop=(k == kc - 1),
                        )
                    nc.scalar.activation(out_sb[:, fo, :], pst, relu,
                                         bias=bias_sb[:, fo:fo + 1])

            f1_sb = consts.tile([128, C, N], BF16, tag="f1_sb")
            node_layer(wa_sb, xT_sb, KA, f1_sb, ba_sb)
            for _ in range(4):
                nc.tensor.matmul(pwarm, warm_sb[:, :128], warm_sb,
                                 start=True, stop=True)
            f2_sb = consts.tile([128, C, N], BF16, tag="f2_sb")
            node_layer(wb_sb, f1_sb, C, f2_sb, bb_sb)

            # ---- G_T = Wcaj_T @ f2_T ; H'_T = Wcai_T @ f2_T + bca --------------
            gt_sb = consts.tile([128, C, N], BF16, tag="gt_sb")
            ht_sb = consts.tile([128, C, N], F32, tag="ht_sb")
            for fo in range(C):
                pst = psmid.tile([128, N], F32, tag="pst", name="pgh")
                for k in range(C):
                    nc.tensor.matmul(
                        pst, wcaj_sb[:, k, fo * 128:(fo + 1) * 128], f2_sb[:, k, :],
                        start=(k == 0), stop=(k == C - 1),
                    )
                nc.scalar.copy(gt_sb[:, fo, :], pst)
                pst2 = psmid.tile([128, N], F32, tag="pst", name="pgh")
                for k in range(C):
                    nc.tensor.matmul(
                        pst2, wcai_sb[:, k, fo * 128:(fo + 1) * 128], f2_sb[:, k, :],
                        start=(k == 0), stop=(k == C - 1),
                    )
                nc.scalar.activation(ht_sb[:, fo, :], pst2, ident,
                                     bias=bca_sb[:, fo:fo + 1])

            # ---- h1 build: per-(chunk, i) DVE tensor_scalar add+relu -----------
            # h1[:, c, il, j] = relu(G[:, c, j] + H'[:, c, g*IG+il]).
            # The per-i addend forces 128-col ops (one per-partition scalar
            # per op); inner-dim broadcast APs measure 2-8x slower, GpSimd
            # measures ~15 ns/col - so 16 narrow DVE ops is the best shape.
            def build_h1(g, eng=None):
                h1 = work.tile([128, C, IG, N], BF16, tag="h1_sb")
                for c in range(C):
                    for il in range(IG):
                        if eng == "act":
                            nc.scalar.activation(
                                h1[:, c, il], gt_sb[:, c, :], relu,
                                bias=ht_sb[:, c, g * IG + il:g * IG + il + 1],
                            )
                        else:
                            nc.vector.tensor_scalar(
                                h1[:, c, il], gt_sb[:, c, :],
                                ht_sb[:, c, g * IG + il:g * IG + il + 1],
                                0.0, add_op, max_op,
                            )
                return h1

            # ---- edge layer helper: 16 matmuls + 4 drains ----------------------
            def edge_layer(w_sb, in_sb, out_sb, bias_sb, drain):
                for fo in range(C):
                    pst = psmid.tile([128, IG * N], F32, tag="pst", name="pedge")
                    for k in range(C):
                        nc.tensor.matmul(
                            pst, w_sb[:, k, fo * 128:(fo + 1) * 128],
                            in_sb[:, k],
                            start=(k == 0), stop=(k == C - 1),
                        )
                    drain(fo, pst, out_sb, bias_sb)

            def drain_act(fo, pst, out_sb, bias_sb):
                nc.scalar.activation(out_sb[:, fo], pst, relu,
                                     bias=bias_sb[:, fo:fo + 1])

            def drain_mixed(fo, pst, out_sb, bias_sb):
                # even fo -> DVE, odd fo -> ACT: the two h3 chunks drain in
                # parallel so the following out matmuls start sooner.
                if fo % 2 == 0:
                    nc.vector.tensor_scalar(
                        out_sb[:, fo], pst, bias_sb[:, fo:fo + 1],
                        0.0, add_op, max_op,
                    )
                else:
                    drain_act(fo, pst, out_sb, bias_sb)

            # ---- out layer: 4-way col-tiled matmuls + partial-sum combine -----
            def emit_out(g, h3_sb):
                # 2-way col tiling: K-chunks {0,1} accumulate at psum
                # partitions 0-1 (array cols 0-31), chunks {2,3} at 32-33
                # (array cols 32-63); the two streams overlap in the PE.
                # Interleaved emission so the second stream issues while the
                # first is still streaming (pending-zero marking is
                # per-partition, so the interleaved start flags are safe).
                pso = psmid.tile([128, IG * N], F32, tag="pst", name="pedge")
                nc.tensor.matmul(pso[0:2, :], wo_sb[:, 0, :], h3_sb[:, 0],
                                 start=True, stop=False, tile_position=(0, 0),
                                 skip_group_check=True)
                nc.tensor.matmul(pso[32:34, :], wo_sb[:, 2, :], h3_sb[:, 2],
                                 start=True, stop=False, tile_position=(0, 32),
                                 skip_group_check=True)
                nc.tensor.matmul(pso[0:2, :], wo_sb[:, 1, :], h3_sb[:, 1],
                                 start=False, stop=True, tile_position=(0, 0),
                                 skip_group_check=True)
                nc.tensor.matmul(pso[32:34, :], wo_sb[:, 3, :], h3_sb[:, 3],
                                 start=False, stop=True, tile_position=(0, 32),
                                 skip_group_check=True)
                # ACT drains both partials immediately (frees the psum bank
                # without waiting on the DVE queue); bo is folded into s1.
                s1 = outp.tile([2, IG * N], F32, tag="s1_sb")
                nc.scalar.activation(s1, pso[0:2, :], ident, bias=bo_sb)
                s2 = outp.tile([2, IG * N], F32, tag="s2_sb")
                nc.scalar.copy(s2, pso[32:34, :])
                return s1, s2

            def emit_out_final(g, s1, s2):
                o_sb = outp.tile([2, IG * N], F32, tag="o_sb")
                nc.vector.tensor_tensor(o_sb, s1, s2, add_op)
                o_v = o_sb.rearrange("p (i j) -> p i j", i=IG)
                nc.sync.dma_start(out=out[:, g * IG:(g + 1) * IG, :], in_=o_v)

            # ---- main pipeline -------------------------------------------------
            # Depth-2 software pipeline: PE order per iteration g is
            #   cb(g+1) -> out(g-1) -> cc(g)
            # so the PE never waits on a drain issued in the same iteration:
            # cc(g) consumes h2(g) drained during the previous iteration, and
            # out(g-1) consumes h3(g-1) drained during cb(g)/cb(g+1).
            h1_t = [None] * NG
            h2_t = [None] * NG
            h3_t = [None] * NG

            h1_t[0] = build_h1(0)
            h1_t[1] = build_h1(1, eng="act")
            h1_t[2] = build_h1(2)
            h2_t[0] = work.tile([128, C, IG, N], BF16, name="h2_sb", tag="h2_sb")
            edge_layer(wcb_sb, h1_t[0], h2_t[0], bcb_sb, drain_act)

            s_prev = None
            for g in range(NG):
                if g + 1 < NG:
                    h2_t[g + 1] = work.tile([128, C, IG, N], BF16, name="h2_sb", tag="h2_sb")
                    edge_layer(wcb_sb, h1_t[g + 1], h2_t[g + 1], bcb_sb,
                               drain_act)
                    h1_t[g + 1] = None
                if g >= 1:
                    s_cur = emit_out(g - 1, h3_t[g - 1])
                    h3_t[g - 1] = None
                if g + 3 < NG:
                    h1_t[g + 3] = build_h1(g + 3)
                h3_t[g] = work.tile([128, C, IG, N], BF16, name="h3_sb", tag="h3_sb")
                edge_layer(wcc_sb, h2_t[g], h3_t[g], bcc_sb, drain_mixed)
                h2_t[g] = None
                if s_prev is not None:
                    emit_out_final(g - 2, *s_prev)
                s_prev = s_cur if g >= 1 else None

            emit_out_final(NG - 2, *s_prev)
            # Last group: sequential accumulation + one biased ACT drain is
            # ~1.4 us shorter on the tail critical path than the tiled
            # matmuls + 3-op combine.
            pso = psmid.tile([128, IG * N], F32, tag="pst", name="pedge")
            for k in range(C):
                nc.tensor.matmul(pso[0:2, :], wo_sb[:, k, :],
                                 h3_t[NG - 1][:, k],
                                 start=(k == 0), stop=(k == C - 1))
            o_sb = outp.tile([2, IG * N], F32, tag="o_sb")
            nc.scalar.activation(o_sb, pso[0:2, :], ident, bias=bo_sb)
            o_v = o_sb.rearrange("p (i j) -> p i j", i=IG)
            nc.sync.dma_start(out=out[:, (NG - 1) * IG:NG * IG, :], in_=o_v)

    nc.compile()
    return nc


def _pack_w(w: np.ndarray) -> np.ndarray:
    """[K, F] f32 -> [128, K//128, F] bf16 so W[k, f] = packed[k % 128, k // 128, f]."""
    k, f = w.shape
    return np.ascontiguousarray(
        w.reshape(k // 128, 128, f).transpose(1, 0, 2)
    ).astype(ml_dtypes.bfloat16)


def _pack_b(b: np.ndarray) -> np.ndarray:
    """[F] f32 -> [128, F//128] f32 so b[f] = packed[f % 128, f // 128]."""
    return np.ascontiguousarray(b.reshape(-1, 128).T).astype(np.float32)


def kernel(brick_vectors, Wa, ba, Wb, bb, Wca, bca, Wcb, bcb, Wcc, bcc, Wo, bo):
    global LAST_RESULTS
    brick_vectors = np.asarray(brick_vectors, dtype=np.float32)

    def u8(a):
        return np.ascontiguousarray(a).view(np.uint8).reshape(128, -1)

    bo_col = np.zeros((128, 1), dtype=np.float32)
    bo_col[:2, 0] = np.asarray(bo, dtype=np.float32).ravel()
    pk2a = np.concatenate([
        u8(_pack_b(np.asarray(bb))), u8(_pack_b(np.asarray(bca))),
        u8(_pack_w(np.asarray(Wb))),
    ], axis=1)
    pk2b = np.concatenate([
        u8(_pack_w(np.asarray(Wca)[:H])), u8(_pack_w(np.asarray(Wca)[H:])),
    ], axis=1)
    pk3a = np.concatenate([
        u8(_pack_b(np.asarray(bcb))), u8(_pack_b(np.asarray(bcc))),
        u8(bo_col), np.zeros((128, 12), dtype=np.uint8),
        u8(_pack_w(np.asarray(Wcb))),
    ], axis=1)
    pk3b = np.concatenate([
        u8(_pack_w(np.asarray(Wcc))), u8(_pack_w(np.asarray(Wo))),
    ], axis=1)
    shared = {"pk2a": pk2a, "pk2b": pk2b, "pk3a": pk3a, "pk3b": pk3b}

    ba_u8 = u8(_pack_b(np.asarray(ba)))
    wa_u8 = u8(_pack_w(np.asarray(Wa)))
    in_maps = []
    for b in range(B):
        xt = _pack_w(brick_vectors[b].T.astype(np.float32))  # [128, KA, N] bf16
        pk1 = np.concatenate([ba_u8, u8(xt), wa_u8], axis=1)
        in_maps.append({"pk1": pk1, **shared})

    nc = _build_nc()
    res = run_bass_kernel_spmd(nc, in_maps, core_ids=list(range(B)))
    LAST_RESULTS = res

    out = np.empty((B, N, N, 2), dtype=np.float32)
    for b in range(B):
        out[b] = res.results[b]["out"].transpose(1, 2, 0)
    return out
